# revision 35
# baseline (speedup 1.0000x reference)
"""Trainium2 kernel for nn_MiddleHeadLayer: 2-layer tanh MLP + row-dot + sigmoid.

    inner = tanh(batch @ W1.T + b1)        batch [N, 1024], W1 [4096, 1024]
    wx    = tanh(inner @ W2.T + b2)        W2 [1024, 4096]
    out   = sigmoid(sum(wx * batch, -1))   [N]

Data-parallel over 8 NeuronCores: each core handles N/8 = 2048 rows;
weights replicated, resident in SBUF as fp16 (fp16 matmuls run at full PE
rate; absmax error ~4e-3, well inside the 2e-2 gate).

Per-core dataflow, in blocks of R=256 rows:
  phase 1: innerT[dff, rows] = tanh(W1T.T @ batchT + b1) — stationary W1T
           chunks [128,128], moving batchT [128, R], fp16 in / f32 PSUM,
           ACT applies the per-partition (d_ff) bias and writes fp16.
  phase 2: wx[rows, dmodel] = tanh(innerT.T @ W2T + b2) — stationary innerT
           chunks, moving W2T [128, 512]. b2 (free-dim bias) is added by a
           DVE tensor_tensor in-place on PSUM (saves 32 rank-1 PE matmuls).
  dot:     z[rows] = sum(wx * batch_f16) via fused DVE scalar_tensor_tensor
           along the free dim; ONE sigmoid + ONE output DMA at the end.

DMA strategy (the baseline's bottleneck): all inputs are pre-packed on the
host into [128, L]-shaped tensors whose per-partition lines are 4-8 KB, so
the whole kernel needs ~28 large DMAs instead of 163 small ones.  Each
dma_start costs ~650 ns of issue time on its engine queue, so issue is
split across three queues (sync: W1/W2 + output, gpsimd: batchT/batch,
scalar: small constants) and ordered just-in-time so the PE's first matmul
can start ~11 us in and never starves.  Eight zero-input warmup matmuls run
during the initial DMA wait to flip the PE HAM clock gate to full rate
before real work arrives.
"""

from contextlib import ExitStack

import numpy as np
import orjson

import concourse.bass as bass
import concourse.tile as tile
from concourse import mybir
from concourse import bass_utils

D_MODEL = 1024
D_FF = 4096
N_TOTAL = 16384
N_CORES = 8
NC_ROWS = N_TOTAL // N_CORES          # 2048 rows per core
# row-block sizes: small first block so the PE can start on ~1 MB of DMA;
# 512-row blocks later halve the per-matmul NX overhead in phase 1
BLOCKS = (256, 256, 512, 512, 512)
K1 = D_MODEL // 128                   # 8 contraction chunks for matmul1
M1 = D_FF // 128                      # 32 d_ff chunks
NH = D_MODEL // 512                   # d_model halves for phase 2 (2)
N_GROUPS = NC_ROWS // 128             # 16
F16 = mybir.dt.float16
F32 = mybir.dt.float32


# ---------------------------------------------------------------------------
# This walrus build rejects >2 sem waits on a single instruction, while Tile's
# wait assignment freely attaches more (e.g. the exit drain gets one wait per
# outstanding logical proc). Legalize at the BIR-JSON level: hoist excess
# waits onto EventSemaphore instructions inserted directly before the
# offending instruction on the same engine stream (identical semantics).
MAX_WAITS = 1


def _legalize_sync_waits(bir: dict) -> dict:
    ctr = 0
    for fn in bir.get("functions", []):
        for blk in fn.get("blocks", []):
            insts = blk.get("instructions")
            if not insts:
                continue
            out = []
            changed = False
            for inst in insts:
                si = inst.get("sync_info")
                ow = (si or {}).get("on_wait") or []
                limit = 2 if inst.get("opcode") == "EventSemaphore" else MAX_WAITS
                if len(ow) > limit:
                    changed = True
                    excess, keep = ow[:-limit], ow[-limit:]
                    for i in range(0, len(excess), 2):
                        ctr += 1
                        out.append({
                            "debug": inst.get("debug"),
                            "engine": inst["engine"],
                            "ins": [],
                            "outs": [],
                            "name": f"legalwait-{ctr}",
                            "opcode": "EventSemaphore",
                            "sync_info": {
                                "on_update": [],
                                "on_wait": excess[i:i + 2],
                            },
                        })
                    si["on_wait"] = keep
                out.append(inst)
            if changed:
                blk["instructions"] = out
    return bir


_orig_to_json_bytes = bass.Bass.to_json_bytes


def _patched_to_json_bytes(self) -> bytes:
    return orjson.dumps(_legalize_sync_waits(orjson.loads(_orig_to_json_bytes(self))))


bass.Bass.to_json_bytes = _patched_to_json_bytes


def build_bass(blocks=BLOCKS):
    nc = bass.Bass("TRN2", target_bir_lowering=False, debug=False)
    n_blocks = len(blocks)
    row0 = [sum(blocks[:i]) for i in range(n_blocks)]   # block start rows

    # Packed DRAM layouts (see _prep_in_maps for the exact packing):
    #  w1p[p, (q*16 + mo2*8 + k)*128 + j] = W1T[k*128+p, (q*2+mo2)*128 + j]
    #  w2p[p, (q*4 + ml)*1024 + c]         = W2T[(q*4+ml)*128 + p, c]
    #  btp[p, (b*8 + k)*256 + r]           = batchT[k*128+p, b*256 + r]
    #  bfp[p, g*1024 + c]                  = batch[g*128+p, c]            (fp16)
    w1p_d = nc.dram_tensor("w1p", [128, 8 * 4096], F16, kind="ExternalInput")
    w2p_d = nc.dram_tensor("w2p", [128, 8 * 4096], F16, kind="ExternalInput")
    btp_d = nc.dram_tensor("btp", [128, 8 * NC_ROWS], F16, kind="ExternalInput")
    bfp_d = nc.dram_tensor("bfp", [128, N_GROUPS * D_MODEL], F16,
                           kind="ExternalInput")
    b1_d = nc.dram_tensor("b1c", [128, M1], F32, kind="ExternalInput")
    b2_d = nc.dram_tensor("b2r", [128, D_MODEL], F32, kind="ExternalInput")
    out_d = nc.dram_tensor("out", [128, N_GROUPS], F32, kind="ExternalOutput")

    n_groups = N_GROUPS

    with tile.TileContext(nc) as tc, ExitStack() as ctx:
        wpool = ctx.enter_context(tc.tile_pool(name="weights", bufs=1))
        btpool = ctx.enter_context(tc.tile_pool(name="batchT", bufs=1))
        bfpool = ctx.enter_context(tc.tile_pool(name="batchf", bufs=1))
        ipool = ctx.enter_context(tc.tile_pool(name="innerT", bufs=36))
        wxpool = ctx.enter_context(tc.tile_pool(name="wx", bufs=4))
        spool = ctx.enter_context(tc.tile_pool(name="scratch", bufs=2))
        zpool = ctx.enter_context(tc.tile_pool(name="z", bufs=1))
        psum1 = ctx.enter_context(tc.tile_pool(name="psum1", bufs=3, space="PSUM"))
        psum2 = ctx.enter_context(tc.tile_pool(name="psum2", bufs=4, space="PSUM"))
        psumw = ctx.enter_context(tc.tile_pool(name="psumw", bufs=1, space="PSUM"))

        # --- small constants on the scalar issue queue (needed by ~12 us) ---
        b1t = wpool.tile([128, M1], F32, tag="b1t")
        nc.scalar.dma_start(b1t[:], b1_d.ap()[:])

        # --- PE warmup: zero matmuls during the initial DMA wait flip the
        # HAM clock gate to 8/8 and bridge until real data lands (~13 us) ---
        warm_s = wpool.tile([128, 128], F16, tag="warm_s")
        warm_m = wpool.tile([128, 512], F16, tag="warm_m")
        nc.vector.memset(warm_s[:], 0.0)
        nc.vector.memset(warm_m[:], 0.0)
        wps = psumw.tile([128, 512], F32)
        for _ in range(10):
            nc.tensor.matmul(wps[:], warm_s[:], warm_m[:], start=True, stop=True,
                             skip_group_check=True)

        # --- everything the critical path needs early goes on the SYNC queue
        # in exact consumption order (DMA engines round-robin across queue
        # ring-sets, so FIFO position within one queue is the only way to
        # prioritize): btp0, then W1 sixteenths, then W2 sixteenths with b2
        # slotted at its need time.  Later batch blocks go on the gpsimd
        # queue where bufs=1 buffer-reuse waits throttle them to exactly
        # when they're needed, keeping ring bandwidth on the weight stream ---
        bt_t, bf_t = [], []

        def emit_bt(b, eng):
            o = 8 * row0[b]
            t = btpool.tile([128, 8 * blocks[b]], F16, tag="bt")
            eng.dma_start(t[:], btp_d.ap()[:, o:o + 8 * blocks[b]])
            bt_t.append(t)

        def emit_bf(b, eng):
            o = (row0[b] // 128) * D_MODEL
            f = bfpool.tile([128, (blocks[b] // 128) * D_MODEL], F16, tag="bf")
            eng.dma_start(f[:], bfp_d.ap()[:, o:o + (blocks[b] // 128) * D_MODEL])
            bf_t.append(f)

        # block-0 batchT split in k-halves so the first matmul group's k=0..3
        # only needs 0.25 MB of batch data (bt0a gets its own pool tag —
        # same-tag bufs=1 would deadlock the k>=4 half)
        R0 = blocks[0]
        bt0a = btpool.tile([128, 4 * R0], F16, tag="bt0a")
        nc.sync.dma_start(bt0a[:], btp_d.ap()[:, 0:4 * R0])
        # w1 tiles: m=0 and m=1 as separate quarter-size DMAs (the first psum
        # group then only waits on 0.5 MB of data), then m-pairs
        w1t = []
        t = wpool.tile([128, 1024], F16, tag="w1m0")
        nc.sync.dma_start(t[:], w1p_d.ap()[:, 0:1024])
        w1t.append(t)
        bt0b = btpool.tile([128, 4 * R0], F16, tag="bt")
        nc.sync.dma_start(bt0b[:], btp_d.ap()[:, 4 * R0:8 * R0])
        t = wpool.tile([128, 1024], F16, tag="w1m1")
        nc.sync.dma_start(t[:], w1p_d.ap()[:, 1024:2048])
        w1t.append(t)
        for q in range(1, 16):
            t = wpool.tile([128, 2048], F16, tag=f"w1q{q}")
            nc.sync.dma_start(t[:], w1p_d.ap()[:, q * 2048:(q + 1) * 2048])
            w1t.append(t)
        bt_t.append(None)  # block 0 handled via bt0a/bt0b
        w2t = []
        b2r = None
        for q in range(16):
            if q == 4:
                b2r = wpool.tile([128, D_MODEL], F32, tag="b2r")
                nc.sync.dma_start(b2r[:], b2_d.ap()[:])
            if q == 6:
                # block-0 dot operand rides the sync FIFO at its need slot
                # (on gpsimd, Tile's readiness-order would run it too early)
                emit_bf(0, nc.sync)
            t = wpool.tile([128, 2048], F16, tag=f"w2q{q}")
            nc.sync.dma_start(t[:], w2p_d.ap()[:, q * 2048:(q + 1) * 2048])
            w2t.append(t)
        if n_blocks > 1:
            emit_bf(1, nc.sync)

        # later batch blocks on the gpsimd queue: every entry carries a
        # bufs=1 buffer-reuse wait, so they self-throttle to exactly when
        # they're needed and never compete with the critical weight stream
        if n_blocks > 1:
            emit_bt(1, nc.gpsimd)
        for b in range(2, n_blocks):
            emit_bt(b, nc.gpsimd)
            emit_bf(b, nc.gpsimd)

        def w1s(m, k):
            # stationary [128, 128] for phase-1 (m, k); w1t[0]/w1t[1] hold
            # m=0/m=1 alone, w1t[i>=2] holds the m-pair (2(i-1), 2(i-1)+1)
            if m < 2:
                return w1t[m][:, k * 128:(k + 1) * 128]
            q, mo2 = divmod(m, 2)
            off = (mo2 * 8 + k) * 128
            return w1t[q + 1][:, off:off + 128]

        def w2s(m, h):
            # moving [128, 512] for phase-2 (m, h)
            q, ml = divmod(m, 2)
            off = ml * 1024 + h * 512
            return w2t[q][:, off:off + 512]

        # per-piece dot partials: column zi*n_groups+g holds the zi-th piece
        # of group g's dot; summed at the end (shortens the tail chain)
        z_h = zpool.tile([128, 3 * n_groups], F32, tag="zh")
        z_all = zpool.tile([128, n_groups], F32)
        sig = zpool.tile([128, n_groups], F32, tag="sig")

        def bt_slice(b, k, Rb):
            if b == 0:
                t = bt0a if k < 4 else bt0b
                return t[:, (k % 4) * Rb:(k % 4 + 1) * Rb]
            return bt_t[b][:, k * Rb:(k + 1) * Rb]

        for b in range(n_blocks):
            Rb = blocks[b]
            # phase 1: innerT chunks [128 dff, Rb rows]
            it = []
            for m in range(M1):
                ps = psum1.tile([128, Rb], F32)
                for k in range(K1):
                    nc.tensor.matmul(
                        ps[:],
                        w1s(m, k),
                        bt_slice(b, k, Rb),
                        start=(k == 0),
                        stop=(k == K1 - 1),
                    )
                t = ipool.tile([128, Rb], F16, tag="it")
                nc.scalar.activation(
                    t[:], ps[:], mybir.ActivationFunctionType.Tanh,
                    bias=b1t[:, m:m + 1],
                )
                it.append(t)

            # phase 2 + row-dot per 128-row group
            for rg in range(Rb // 128):
                g = row0[b] // 128 + rg
                last_grp = (b == n_blocks - 1) and (rg == Rb // 128 - 1)
                wx = wxpool.tile([128, D_MODEL], F16, tag="wx")
                # the very last group's trailing half runs as narrowing psum
                # groups so the end-of-kernel DVE/ACT chain is shortest
                pieces = [(0, 512), (512, 512)] if not last_grp else \
                         [(0, 512), (512, 384), (896, 128)]
                for zi, (c0, cw) in enumerate(pieces):
                    ps2 = psum2.tile([128, cw], F32)
                    for m in range(M1):
                        nc.tensor.matmul(
                            ps2[:],
                            it[m][:, rg * 128:(rg + 1) * 128],
                            w2s(m, c0 // 512)[:, c0 % 512:c0 % 512 + cw],
                            start=(m == 0),
                            stop=(m == M1 - 1),
                        )
                    # b2 (free-dim bias): DVE add in-place on PSUM
                    nc.vector.tensor_tensor(
                        ps2[:], ps2[:], b2r[:, c0:c0 + cw],
                        mybir.AluOpType.add,
                    )
                    nc.scalar.activation(
                        wx[:, c0:c0 + cw], ps2[:],
                        mybir.ActivationFunctionType.Tanh,
                    )
                    # z piece = sum(wx * batch) over these cols (DVE);
                    # piece zi accumulates into z_h column zi*n_groups+g
                    scratch = spool.tile([128, cw], F16, tag="scr")
                    nc.vector.scalar_tensor_tensor(
                        out=scratch[:],
                        in0=wx[:, c0:c0 + cw],
                        scalar=1.0,
                        in1=bf_t[b][:, rg * D_MODEL + c0:rg * D_MODEL + c0 + cw],
                        op0=mybir.AluOpType.mult,
                        op1=mybir.AluOpType.mult,
                        accum_out=z_h[:, zi * n_groups + g:zi * n_groups + g + 1],
                    )

                if g == n_groups - 2:
                    # groups 0..14 are final: sum, sigmoid and ship them now
                    # so their DMA completion overlaps the last group's MMs
                    nc.vector.tensor_tensor(
                        z_all[:, 0:n_groups - 1], z_h[:, 0:n_groups - 1],
                        z_h[:, n_groups:2 * n_groups - 1],
                        mybir.AluOpType.add,
                    )
                    nc.scalar.activation(
                        sig[:, 0:n_groups - 1], z_all[:, 0:n_groups - 1],
                        mybir.ActivationFunctionType.Sigmoid,
                    )
                    nc.sync.dma_start(
                        out_d.ap()[:, 0:n_groups - 1], sig[:, 0:n_groups - 1])

        # last group: z = zh0 + zh1 + zh2, tiny sigmoid + tiny output DMA
        gl = n_groups - 1
        nc.vector.tensor_tensor(
            z_all[:, gl:gl + 1], z_h[:, gl:gl + 1],
            z_h[:, n_groups + gl:n_groups + gl + 1],
            mybir.AluOpType.add,
        )
        nc.vector.tensor_tensor(
            z_all[:, gl:gl + 1], z_all[:, gl:gl + 1],
            z_h[:, 2 * n_groups + gl:2 * n_groups + gl + 1],
            mybir.AluOpType.add,
        )
        nc.scalar.activation(
            sig[:, gl:gl + 1], z_all[:, gl:gl + 1],
            mybir.ActivationFunctionType.Sigmoid,
        )
        nc.sync.dma_start(out_d.ap()[:, gl:gl + 1], sig[:, gl:gl + 1])

    return nc


_CACHED = {}


def _get_nc(blocks=BLOCKS):
    if blocks not in _CACHED:
        _CACHED[blocks] = build_bass(blocks)
    return _CACHED[blocks]


def _prep_in_maps(batch, W1, b1, W2, b2):
    batch = np.ascontiguousarray(batch, dtype=np.float32)
    w1t = W1.T.astype(np.float16)                           # [1024, 4096]
    w2t = W2.T.astype(np.float16)                           # [4096, 1024]

    # w1p: [p, q, mo2, k, j] with m = q*2 + mo2
    #   A[k, p, m, j] -> [p, m(=32), k, j] -> split m into (16, 2) -> pack
    A = w1t.reshape(K1, 128, M1, 128).transpose(1, 2, 0, 3)   # [p, m, k, j]
    w1p = np.ascontiguousarray(
        A.reshape(128, 16, 2, K1, 128).reshape(128, 8 * 4096))

    # w2p: [p, q, ml, c] with m = q*2 + ml
    C = w2t.reshape(M1, 128, D_MODEL).transpose(1, 0, 2)      # [p, m, c]
    w2p = np.ascontiguousarray(C.reshape(128, 8 * 4096))
    # (m-major layout is identical for any even split; slicing handles q)

    # b1 as [128, 32]: column m holds b1[m*128:(m+1)*128] (per-partition bias)
    b1c = np.ascontiguousarray(
        np.asarray(b1, dtype=np.float32).reshape(M1, 128).T)
    # b2 replicated across partitions for the DVE free-dim bias add
    b2r = np.ascontiguousarray(
        np.broadcast_to(np.asarray(b2, dtype=np.float32)[None, :],
                        (128, D_MODEL)))

    batcht = batch.T.astype(np.float16)                       # [1024, 16384]
    batch16 = batch.astype(np.float16)                        # [16384, 1024]

    in_maps = []
    for c in range(N_CORES):
        r0, r1 = c * NC_ROWS, (c + 1) * NC_ROWS
        # btp: per-block slabs, each [p, k, r] flattened
        bct = batcht[:, r0:r1]
        slabs, off = [], 0
        for Rb in BLOCKS:
            s = bct[:, off:off + Rb].reshape(K1, 128, Rb).transpose(1, 0, 2)
            slabs.append(s.reshape(128, K1 * Rb))
            off += Rb
        btp = np.ascontiguousarray(np.concatenate(slabs, axis=1))
        # bfp: [p, g, c]
        E = batch16[r0:r1].reshape(N_GROUPS, 128, D_MODEL).transpose(1, 0, 2)
        bfp = np.ascontiguousarray(E.reshape(128, N_GROUPS * D_MODEL))
        in_maps.append({
            "w1p": w1p,
            "w2p": w2p,
            "b1c": b1c,
            "b2r": b2r,
            "btp": btp,
            "bfp": bfp,
        })
    return in_maps


def kernel(batch, W1, b1, W2, b2, _trace=False, _trace_kwargs=None):
    in_maps = _prep_in_maps(batch, W1, b1, W2, b2)
    nc = _get_nc()
    res = bass_utils.run_bass_kernel_spmd(
        nc, in_maps, core_ids=list(range(N_CORES)),
        trace=_trace, **(_trace_kwargs or {}),
    )
    # out[p, g] holds row g*128+p of the core's 2048 rows
    out = np.concatenate([
        np.ascontiguousarray(res.results[c]["out"].T).reshape(-1)
        for c in range(N_CORES)
    ])
    if _trace:
        return out, res
    return out


# revision 38
# speedup vs baseline: 1.0063x; 1.0063x over previous
"""Trainium2 kernel for nn_MiddleHeadLayer: 2-layer tanh MLP + row-dot + sigmoid.

    inner = tanh(batch @ W1.T + b1)        batch [N, 1024], W1 [4096, 1024]
    wx    = tanh(inner @ W2.T + b2)        W2 [1024, 4096]
    out   = sigmoid(sum(wx * batch, -1))   [N]

Data-parallel over 8 NeuronCores: each core handles N/8 = 2048 rows;
weights replicated, resident in SBUF as fp16 (fp16 matmuls run at full PE
rate; absmax error ~4e-3, well inside the 2e-2 gate).

Per-core dataflow, in blocks of R=256 rows:
  phase 1: innerT[dff, rows] = tanh(W1T.T @ batchT + b1) — stationary W1T
           chunks [128,128], moving batchT [128, R], fp16 in / f32 PSUM,
           ACT applies the per-partition (d_ff) bias and writes fp16.
  phase 2: wx[rows, dmodel] = tanh(innerT.T @ W2T + b2) — stationary innerT
           chunks, moving W2T [128, 512]. b2 (free-dim bias) is added by a
           DVE tensor_tensor in-place on PSUM (saves 32 rank-1 PE matmuls).
  dot:     z[rows] = sum(wx * batch_f16) via fused DVE scalar_tensor_tensor
           along the free dim; ONE sigmoid + ONE output DMA at the end.

DMA strategy (the baseline's bottleneck): all inputs are pre-packed on the
host into [128, L]-shaped tensors whose per-partition lines are 4-8 KB, so
the whole kernel needs ~28 large DMAs instead of 163 small ones.  Each
dma_start costs ~650 ns of issue time on its engine queue, so issue is
split across three queues (sync: W1/W2 + output, gpsimd: batchT/batch,
scalar: small constants) and ordered just-in-time so the PE's first matmul
can start ~11 us in and never starves.  Eight zero-input warmup matmuls run
during the initial DMA wait to flip the PE HAM clock gate to full rate
before real work arrives.
"""

from contextlib import ExitStack

import numpy as np
import orjson

import concourse.bass as bass
import concourse.tile as tile
from concourse import mybir
from concourse import bass_utils

D_MODEL = 1024
D_FF = 4096
N_TOTAL = 16384
N_CORES = 8
NC_ROWS = N_TOTAL // N_CORES          # 2048 rows per core
# row-block sizes: small first block so the PE can start on ~1 MB of DMA;
# 512-row blocks later halve the per-matmul NX overhead in phase 1
BLOCKS = (256, 256, 512, 512, 512)
K1 = D_MODEL // 128                   # 8 contraction chunks for matmul1
M1 = D_FF // 128                      # 32 d_ff chunks
NH = D_MODEL // 512                   # d_model halves for phase 2 (2)
N_GROUPS = NC_ROWS // 128             # 16
F16 = mybir.dt.float16
F32 = mybir.dt.float32


# ---------------------------------------------------------------------------
# This walrus build rejects >2 sem waits on a single instruction, while Tile's
# wait assignment freely attaches more (e.g. the exit drain gets one wait per
# outstanding logical proc). Legalize at the BIR-JSON level: hoist excess
# waits onto EventSemaphore instructions inserted directly before the
# offending instruction on the same engine stream (identical semantics).
MAX_WAITS = 1


def _legalize_sync_waits(bir: dict) -> dict:
    ctr = 0
    for fn in bir.get("functions", []):
        for blk in fn.get("blocks", []):
            insts = blk.get("instructions")
            if not insts:
                continue
            out = []
            changed = False
            for inst in insts:
                si = inst.get("sync_info")
                ow = (si or {}).get("on_wait") or []
                limit = 2 if inst.get("opcode") == "EventSemaphore" else MAX_WAITS
                if len(ow) > limit:
                    changed = True
                    excess, keep = ow[:-limit], ow[-limit:]
                    for i in range(0, len(excess), 2):
                        ctr += 1
                        out.append({
                            "debug": inst.get("debug"),
                            "engine": inst["engine"],
                            "ins": [],
                            "outs": [],
                            "name": f"legalwait-{ctr}",
                            "opcode": "EventSemaphore",
                            "sync_info": {
                                "on_update": [],
                                "on_wait": excess[i:i + 2],
                            },
                        })
                    si["on_wait"] = keep
                out.append(inst)
            if changed:
                blk["instructions"] = out
    return bir


_orig_to_json_bytes = bass.Bass.to_json_bytes


def _patched_to_json_bytes(self) -> bytes:
    return orjson.dumps(_legalize_sync_waits(orjson.loads(_orig_to_json_bytes(self))))


bass.Bass.to_json_bytes = _patched_to_json_bytes


def build_bass(blocks=BLOCKS):
    nc = bass.Bass("TRN2", target_bir_lowering=False, debug=False)
    n_blocks = len(blocks)
    row0 = [sum(blocks[:i]) for i in range(n_blocks)]   # block start rows

    # Packed DRAM layouts (see _prep_in_maps for the exact packing):
    #  w1p[p, (q*16 + mo2*8 + k)*128 + j] = W1T[k*128+p, (q*2+mo2)*128 + j]
    #  w2p[p, (q*4 + ml)*1024 + c]         = W2T[(q*4+ml)*128 + p, c]
    #  btp[p, (b*8 + k)*256 + r]           = batchT[k*128+p, b*256 + r]
    #  bfp[p, g*1024 + c]                  = batch[g*128+p, c]            (fp16)
    w1p_d = nc.dram_tensor("w1p", [128, 8 * 4096], F16, kind="ExternalInput")
    w2p_d = nc.dram_tensor("w2p", [128, 8 * 4096], F16, kind="ExternalInput")
    btp_d = nc.dram_tensor("btp", [128, 8 * NC_ROWS], F16, kind="ExternalInput")
    bfp_d = nc.dram_tensor("bfp", [128, N_GROUPS * D_MODEL], F16,
                           kind="ExternalInput")
    b1_d = nc.dram_tensor("b1c", [128, M1], F32, kind="ExternalInput")
    b2_d = nc.dram_tensor("b2r", [128, D_MODEL], F32, kind="ExternalInput")
    out_d = nc.dram_tensor("out", [128, N_GROUPS], F32, kind="ExternalOutput")

    n_groups = N_GROUPS

    with tile.TileContext(nc) as tc, ExitStack() as ctx:
        wpool = ctx.enter_context(tc.tile_pool(name="weights", bufs=1))
        btpool = ctx.enter_context(tc.tile_pool(name="batchT", bufs=1))
        bfpool = ctx.enter_context(tc.tile_pool(name="batchf", bufs=1))
        ipool = ctx.enter_context(tc.tile_pool(name="innerT", bufs=36))
        wxpool = ctx.enter_context(tc.tile_pool(name="wx", bufs=4))
        spool = ctx.enter_context(tc.tile_pool(name="scratch", bufs=2))
        zpool = ctx.enter_context(tc.tile_pool(name="z", bufs=1))
        psum1 = ctx.enter_context(tc.tile_pool(name="psum1", bufs=3, space="PSUM"))
        psum2 = ctx.enter_context(tc.tile_pool(name="psum2", bufs=4, space="PSUM"))
        psumw = ctx.enter_context(tc.tile_pool(name="psumw", bufs=1, space="PSUM"))

        # --- small constants on the scalar issue queue (needed by ~12 us) ---
        b1t = wpool.tile([128, M1], F32, tag="b1t")
        nc.scalar.dma_start(b1t[:], b1_d.ap()[:])

        # --- PE warmup: zero matmuls during the initial DMA wait flip the
        # HAM clock gate to 8/8 and bridge until real data lands (~13 us) ---
        warm_s = wpool.tile([128, 128], F16, tag="warm_s")
        warm_m = wpool.tile([128, 512], F16, tag="warm_m")
        nc.vector.memset(warm_s[:], 0.0)
        nc.vector.memset(warm_m[:], 0.0)
        wps = psumw.tile([128, 512], F32)
        for _ in range(11):
            nc.tensor.matmul(wps[:], warm_s[:], warm_m[:], start=True, stop=True,
                             skip_group_check=True)

        # --- everything the critical path needs early goes on the SYNC queue
        # in exact consumption order (DMA engines round-robin across queue
        # ring-sets, so FIFO position within one queue is the only way to
        # prioritize): btp0, then W1 sixteenths, then W2 sixteenths with b2
        # slotted at its need time.  Later batch blocks go on the gpsimd
        # queue where bufs=1 buffer-reuse waits throttle them to exactly
        # when they're needed, keeping ring bandwidth on the weight stream ---
        bt_t, bf_t = [], []

        def emit_bt(b, eng):
            o = 8 * row0[b]
            t = btpool.tile([128, 8 * blocks[b]], F16, tag="bt")
            eng.dma_start(t[:], btp_d.ap()[:, o:o + 8 * blocks[b]])
            bt_t.append(t)

        def emit_bf(b, eng):
            o = (row0[b] // 128) * D_MODEL
            f = bfpool.tile([128, (blocks[b] // 128) * D_MODEL], F16, tag="bf")
            eng.dma_start(f[:], bfp_d.ap()[:, o:o + (blocks[b] // 128) * D_MODEL])
            bf_t.append(f)

        # block-0 batchT split in k-halves so the first matmul group's k=0..3
        # only needs 0.25 MB of batch data (bt0a gets its own pool tag —
        # same-tag bufs=1 would deadlock the k>=4 half)
        R0 = blocks[0]
        bt0a = btpool.tile([128, 4 * R0], F16, tag="bt0a")
        nc.sync.dma_start(bt0a[:], btp_d.ap()[:, 0:4 * R0])
        # w1 tiles: m=0 and m=1 as separate quarter-size DMAs (the first psum
        # group then only waits on 0.5 MB of data), then m-pairs
        w1t = []
        bt0b = None
        for q in range(16):
            t = wpool.tile([128, 2048], F16, tag=f"w1q{q}")
            nc.sync.dma_start(t[:], w1p_d.ap()[:, q * 2048:(q + 1) * 2048])
            w1t.append(t)
            if q == 0:
                bt0b = btpool.tile([128, 4 * R0], F16, tag="bt")
                nc.sync.dma_start(bt0b[:], btp_d.ap()[:, 4 * R0:8 * R0])
        bt_t.append(None)  # block 0 handled via bt0a/bt0b
        w2t = []
        b2r = None
        for q in range(16):
            if q == 4:
                b2r = wpool.tile([128, D_MODEL], F32, tag="b2r")
                nc.sync.dma_start(b2r[:], b2_d.ap()[:])
            if q == 6:
                # block-0 dot operand rides the sync FIFO at its need slot
                # (on gpsimd, Tile's readiness-order would run it too early)
                emit_bf(0, nc.sync)
            t = wpool.tile([128, 2048], F16, tag=f"w2q{q}")
            nc.sync.dma_start(t[:], w2p_d.ap()[:, q * 2048:(q + 1) * 2048])
            w2t.append(t)
        if n_blocks > 1:
            emit_bf(1, nc.sync)

        # later batch blocks on the gpsimd queue: every entry carries a
        # bufs=1 buffer-reuse wait, so they self-throttle to exactly when
        # they're needed and never compete with the critical weight stream
        if n_blocks > 1:
            emit_bt(1, nc.gpsimd)
        for b in range(2, n_blocks):
            emit_bt(b, nc.gpsimd)
            emit_bf(b, nc.gpsimd)

        def w1s(m, k):
            # stationary [128, 128] for phase-1 (m, k)
            q, mo2 = divmod(m, 2)
            off = (mo2 * 8 + k) * 128
            return w1t[q][:, off:off + 128]

        def w2s(m, h):
            # moving [128, 512] for phase-2 (m, h)
            q, ml = divmod(m, 2)
            off = ml * 1024 + h * 512
            return w2t[q][:, off:off + 512]

        # per-piece dot partials: column zi*n_groups+g holds the zi-th piece
        # of group g's dot; summed at the end (shortens the tail chain)
        z_h = zpool.tile([128, 3 * n_groups], F32, tag="zh")
        z_all = zpool.tile([128, n_groups], F32)
        sig = zpool.tile([128, n_groups], F32, tag="sig")

        def bt_slice(b, k, Rb):
            if b == 0:
                t = bt0a if k < 4 else bt0b
                return t[:, (k % 4) * Rb:(k % 4 + 1) * Rb]
            return bt_t[b][:, k * Rb:(k + 1) * Rb]

        for b in range(n_blocks):
            Rb = blocks[b]
            # phase 1: innerT chunks [128 dff, Rb rows]
            it = []
            for m in range(M1):
                ps = psum1.tile([128, Rb], F32)
                for k in range(K1):
                    nc.tensor.matmul(
                        ps[:],
                        w1s(m, k),
                        bt_slice(b, k, Rb),
                        start=(k == 0),
                        stop=(k == K1 - 1),
                    )
                t = ipool.tile([128, Rb], F16, tag="it")
                nc.scalar.activation(
                    t[:], ps[:], mybir.ActivationFunctionType.Tanh,
                    bias=b1t[:, m:m + 1],
                )
                it.append(t)

            # phase 2 + row-dot per 128-row group
            for rg in range(Rb // 128):
                g = row0[b] // 128 + rg
                last_grp = (b == n_blocks - 1) and (rg == Rb // 128 - 1)
                wx = wxpool.tile([128, D_MODEL], F16, tag="wx")
                # the very last group's trailing half runs as narrowing psum
                # groups so the end-of-kernel DVE/ACT chain is shortest
                pieces = [(0, 512), (512, 512)] if not last_grp else \
                         [(0, 512), (512, 384), (896, 128)]
                for zi, (c0, cw) in enumerate(pieces):
                    ps2 = psum2.tile([128, cw], F32)
                    for m in range(M1):
                        nc.tensor.matmul(
                            ps2[:],
                            it[m][:, rg * 128:(rg + 1) * 128],
                            w2s(m, c0 // 512)[:, c0 % 512:c0 % 512 + cw],
                            start=(m == 0),
                            stop=(m == M1 - 1),
                        )
                    # b2 (free-dim bias): DVE add in-place on PSUM
                    nc.vector.tensor_tensor(
                        ps2[:], ps2[:], b2r[:, c0:c0 + cw],
                        mybir.AluOpType.add,
                    )
                    nc.scalar.activation(
                        wx[:, c0:c0 + cw], ps2[:],
                        mybir.ActivationFunctionType.Tanh,
                    )
                    # z piece = sum(wx * batch) over these cols (DVE);
                    # piece zi accumulates into z_h column zi*n_groups+g
                    scratch = spool.tile([128, cw], F16, tag="scr")
                    nc.vector.scalar_tensor_tensor(
                        out=scratch[:],
                        in0=wx[:, c0:c0 + cw],
                        scalar=1.0,
                        in1=bf_t[b][:, rg * D_MODEL + c0:rg * D_MODEL + c0 + cw],
                        op0=mybir.AluOpType.mult,
                        op1=mybir.AluOpType.mult,
                        accum_out=z_h[:, zi * n_groups + g:zi * n_groups + g + 1],
                    )

                if g == n_groups - 2:
                    # groups 0..14 are final: sum, sigmoid and ship them now
                    # so their DMA completion overlaps the last group's MMs
                    nc.vector.tensor_tensor(
                        z_all[:, 0:n_groups - 1], z_h[:, 0:n_groups - 1],
                        z_h[:, n_groups:2 * n_groups - 1],
                        mybir.AluOpType.add,
                    )
                    nc.scalar.activation(
                        sig[:, 0:n_groups - 1], z_all[:, 0:n_groups - 1],
                        mybir.ActivationFunctionType.Sigmoid,
                    )
                    nc.sync.dma_start(
                        out_d.ap()[:, 0:n_groups - 1], sig[:, 0:n_groups - 1])

        # last group: z = zh0 + zh1 + zh2, tiny sigmoid + tiny output DMA
        gl = n_groups - 1
        nc.vector.tensor_tensor(
            z_all[:, gl:gl + 1], z_h[:, gl:gl + 1],
            z_h[:, n_groups + gl:n_groups + gl + 1],
            mybir.AluOpType.add,
        )
        nc.vector.tensor_tensor(
            z_all[:, gl:gl + 1], z_all[:, gl:gl + 1],
            z_h[:, 2 * n_groups + gl:2 * n_groups + gl + 1],
            mybir.AluOpType.add,
        )
        nc.scalar.activation(
            sig[:, gl:gl + 1], z_all[:, gl:gl + 1],
            mybir.ActivationFunctionType.Sigmoid,
        )
        nc.sync.dma_start(out_d.ap()[:, gl:gl + 1], sig[:, gl:gl + 1])

    return nc


_CACHED = {}


def _get_nc(blocks=BLOCKS):
    if blocks not in _CACHED:
        _CACHED[blocks] = build_bass(blocks)
    return _CACHED[blocks]


def _prep_in_maps(batch, W1, b1, W2, b2):
    batch = np.ascontiguousarray(batch, dtype=np.float32)
    w1t = W1.T.astype(np.float16)                           # [1024, 4096]
    w2t = W2.T.astype(np.float16)                           # [4096, 1024]

    # w1p: [p, q, mo2, k, j] with m = q*2 + mo2
    #   A[k, p, m, j] -> [p, m(=32), k, j] -> split m into (16, 2) -> pack
    A = w1t.reshape(K1, 128, M1, 128).transpose(1, 2, 0, 3)   # [p, m, k, j]
    w1p = np.ascontiguousarray(
        A.reshape(128, 16, 2, K1, 128).reshape(128, 8 * 4096))

    # w2p: [p, q, ml, c] with m = q*2 + ml
    C = w2t.reshape(M1, 128, D_MODEL).transpose(1, 0, 2)      # [p, m, c]
    w2p = np.ascontiguousarray(C.reshape(128, 8 * 4096))
    # (m-major layout is identical for any even split; slicing handles q)

    # b1 as [128, 32]: column m holds b1[m*128:(m+1)*128] (per-partition bias)
    b1c = np.ascontiguousarray(
        np.asarray(b1, dtype=np.float32).reshape(M1, 128).T)
    # b2 replicated across partitions for the DVE free-dim bias add
    b2r = np.ascontiguousarray(
        np.broadcast_to(np.asarray(b2, dtype=np.float32)[None, :],
                        (128, D_MODEL)))

    batcht = batch.T.astype(np.float16)                       # [1024, 16384]
    batch16 = batch.astype(np.float16)                        # [16384, 1024]

    in_maps = []
    for c in range(N_CORES):
        r0, r1 = c * NC_ROWS, (c + 1) * NC_ROWS
        # btp: per-block slabs, each [p, k, r] flattened
        bct = batcht[:, r0:r1]
        slabs, off = [], 0
        for Rb in BLOCKS:
            s = bct[:, off:off + Rb].reshape(K1, 128, Rb).transpose(1, 0, 2)
            slabs.append(s.reshape(128, K1 * Rb))
            off += Rb
        btp = np.ascontiguousarray(np.concatenate(slabs, axis=1))
        # bfp: [p, g, c]
        E = batch16[r0:r1].reshape(N_GROUPS, 128, D_MODEL).transpose(1, 0, 2)
        bfp = np.ascontiguousarray(E.reshape(128, N_GROUPS * D_MODEL))
        in_maps.append({
            "w1p": w1p,
            "w2p": w2p,
            "b1c": b1c,
            "b2r": b2r,
            "btp": btp,
            "bfp": bfp,
        })
    return in_maps


def kernel(batch, W1, b1, W2, b2, _trace=False, _trace_kwargs=None):
    in_maps = _prep_in_maps(batch, W1, b1, W2, b2)
    nc = _get_nc()
    res = bass_utils.run_bass_kernel_spmd(
        nc, in_maps, core_ids=list(range(N_CORES)),
        trace=_trace, **(_trace_kwargs or {}),
    )
    # out[p, g] holds row g*128+p of the core's 2048 rows
    out = np.concatenate([
        np.ascontiguousarray(res.results[c]["out"].T).reshape(-1)
        for c in range(N_CORES)
    ])
    if _trace:
        return out, res
    return out
